# revision 50
# baseline (speedup 1.0000x reference)
"""MoE top-2 routing kernel for Trainium2 (8 NeuronCores, merged-pair).

Key algebraic trick: the reference combine is an UNWEIGHTED mean of the
two selected experts, so for every token
    out = 0.5*(x @ We1 + x @ We2) + 0.5*(be1 + be2)
        = x @ (0.5*(We1 + We2)) + 0.5*(be1 + be2).
Tokens sharing the same top-2 pair (45 distinct pairs for E=10) need only
ONE matmul against the host-pre-merged pair weight — half the PE work of
per-expert dispatch. The kernel is then DMA-bound on streaming the merged
pair weights (~45 x 1-2MB over 8 cores), and every structural choice
below serves keeping that stream at the HBM rate with the PE fed.

Orientation: "transposed" streaming. Stationary operand = 128x128 W
blocks in natural [K, F] layout; moving operand = x^T columns (tokens).
PE cost is 64*T cycles per pair (T = token count) with NO padding to
128-token tiles, and x/out DMA carry no padding either.

SPMD: one program for all 8 cores, so the slot structure is rank-uniform:
S=6 slots per core (45 pairs + 3 split halves = 48 pieces, snake-dealt
by size), rank r padded to a common width prof[r] across cores.

Precision budget: bf16 x and bf16 weights give ~2.3e-3 rel err; the 3
token-smallest ranks store weights as fp8e4m3 (x64 host-side scale to
dodge denormals, undone on host) for ~1.76e-2 total — inside the 2e-2
gate — and cut the weight stream by 3MB/core. The PE accepts mixed
fp8-stationary x bf16-moving matmuls at full rate (HW-verified).

Schedule (all slots' weights stay resident in SBUF, ~9MB):
  sync ring:   per slot in compute order: x^T slice, then the slot's
               weight sub-DMAs (quarters for bf16, halves for fp8 — all
               4KB/partition descriptors = full DMA rate). FIFO position
               IS the prefetch schedule; the ring never waits.
  scalar ring: x^T slice 0 (parallel head), then per-slot out stores
               gated on the casts; the final store is split in half to
               overlap the last matmul groups.
  tensor:      N=512 garbage warm-up matmuls sized to park ALL of the
               PE's stream-lag idle at the head (any mid-stream idle
               risks a >3.4us HAM re-throttle to 1.2GHz); then per slot
               8 f-groups x 8 ki matmuls, psum bank = f.
  vector:      psum -> bf16 casts into rotating out buffers.
  gpsimd:      semaphore reset up front.
Semaphore traffic is kept rare (every wait/inc costs ~115ns of serial
drain at block exit): one sem per DMA (completion counters on shared
lanes can interleave — never share), casts/psum-reuse gated per
half-slot, matmul groups signalled per pair of f-groups.

Host does routing (exactly the reference's jax ops — tie-breaking must
match bit-for-bit), pair merging, packing, gather/scatter, bias add.
"""

import os
from contextlib import ExitStack

import ml_dtypes
import numpy as np

import concourse.bass as bass
import concourse.mybir as mybir
from concourse.bass_utils import run_bass_kernel_spmd

N = 8192
D = 1024
E = 10
TOP_K = 2
P = 128
KC = 8   # contraction chunks of 128
FC = 8   # output-feature chunks of 128
NCORES = 8
BF16 = ml_dtypes.bfloat16

_last_results = None  # stash for test harness (exec_time_ns etc.)
_prog_cache = {}


def _route(x, Wr, br):
    """Top-2 expert ids per token, replicating reference ops exactly."""
    import jax
    import jax.numpy as jnp

    logits = jnp.asarray(x) @ jnp.asarray(Wr).T + jnp.asarray(br)
    probs = jax.nn.softmax(logits, axis=-1)
    _, idx = jax.lax.top_k(probs, TOP_K)
    return np.asarray(idx)


def _pack(pieces):
    """pieces: list of (pid, tok_array). Split/pad to exactly 8*S pieces
    (S >= ceil/8), snake-deal sorted-desc into an 8 x S grid, and return
    (grid, prof): grid[c][r] = (pid, toks), prof[r] = common padded width
    of rank r (multiple of 4, >= 4, <= 512)."""
    pieces = [(pid, t) for pid, t in pieces if len(t) > 0]
    # psum bank limit: T <= 512
    changed = True
    while changed:
        changed = False
        for i, (pid, t) in enumerate(pieces):
            if len(t) > 512:
                h = len(t) // 2
                pieces[i] = (pid, t[:h])
                pieces.append((pid, t[h:]))
                changed = True
    S = max(1, -(-len(pieces) // 8))
    need = 8 * S - len(pieces)
    if need > 0 and sum(len(t) for _, t in pieces) >= 8 * S:
        # Choose which `need` pieces to half-split to minimize
        # sum-of-rank-maxima of the sorted result (the padded profile).
        def profile_cost(vals):
            vals = sorted(vals, reverse=True)
            return sum(vals[8 * r] for r in range(S))

        import itertools

        cand = list(range(len(pieces)))
        best, best_cost = None, None
        # limit search: only the 16 largest pieces are worth splitting
        pool = sorted(cand, key=lambda i: -len(pieces[i][1]))[:16]
        for combo in itertools.combinations(pool, need):
            vals = []
            for i in cand:
                n = len(pieces[i][1])
                if i in combo:
                    vals += [n - n // 2, n // 2]
                else:
                    vals.append(n)
            c = profile_cost(vals)
            if best_cost is None or c < best_cost:
                best, best_cost = combo, c
        for i in best:
            pid, t = pieces[i]
            h = len(t) // 2
            pieces[i] = (pid, t[:h])
            pieces.append((pid, t[h:]))
    else:
        while len(pieces) < 8 * S:
            pieces.sort(key=lambda p: -len(p[1]))
            pid, t = pieces[0]
            if len(t) >= 2:
                h = len(t) // 2
                pieces[0] = (pid, t[:h])
                pieces.append((pid, t[h:]))
            else:
                pieces.append((-1, np.zeros(0, dtype=np.int64)))
    pieces.sort(key=lambda p: -len(p[1]))
    grid = [[None] * S for _ in range(NCORES)]
    for r in range(S):
        row = pieces[8 * r : 8 * r + 8]
        order = range(NCORES) if r % 2 == 0 else range(NCORES - 1, -1, -1)
        for k, c in enumerate(order):
            grid[c][r] = row[k]
    prof = []
    for r in range(S):
        mx = max(len(grid[c][r][1]) for c in range(NCORES))
        prof.append(max(4, -(-mx // 4) * 4))
    return grid, prof


def _build_program(prof, is8):
    """Raw-bass SPMD program: S weight slots of common rank widths prof.

    Engines: sync = weight half-DMAs (HWDGE), scalar = x loads + out
    stores (HWDGE), tensor = warmup + 8 f-groups x 8 ki matmuls per slot,
    vector = psum->sbuf bf16 casts, gpsimd = semaphore reset up front.
    """
    S = len(prof)
    sumT = sum(prof)
    Xoff = [0]
    for t in prof:
        Xoff.append(Xoff[-1] + 8 * t)
    bf16 = mybir.dt.bfloat16
    f32 = mybir.dt.float32
    # All weight slots resident (S*2MB = 12MB SBUF): the weight stream
    # runs gate-free start-to-finish, so the DMA queue never stalls on
    # the PE and the PE never re-throttles waiting for weights.
    WBUF = S
    OB = min(3, S)
    WARM = int(os.environ.get("KERNEL_WARM", "38"))
    f8 = mybir.dt.float8e4
    n8 = sum(is8)
    nbf = S - n8
    # Weight sub-DMA granularity: quarters for bf16 slots, halves for
    # fp8 — both 4KB/partition descriptors (full DMA rate), and the PE
    # is released per 2 f-chunks, so stream-vs-PE slack never pools into
    # a >3.4us idle gap (which would trip the HAM clock gate).
    NQ = [2 if is8[j] else 4 for j in range(S)]
    wbase = [0]
    for j in range(S):
        wbase.append(wbase[-1] + NQ[j])
    # DRAM index of slot j within its dtype-group tensor
    gidx = []
    c_bf = c_f8 = 0
    for j in range(S):
        if is8[j]:
            gidx.append(c_f8)
            c_f8 += 1
        else:
            gidx.append(c_bf)
            c_bf += 1

    nc = bass.Bass("TRN2", target_bir_lowering=False, debug=False)
    xT = nc.dram_tensor("xT", [P, 8 * sumT], bf16, kind="ExternalInput")
    w = nc.dram_tensor("w", [max(nbf, 1), P, KC * D], bf16, kind="ExternalInput")
    if n8:
        w8 = nc.dram_tensor("w8", [n8, P, KC * D], f8, kind="ExternalInput")
    out = nc.dram_tensor("out", [P, 8 * sumT], bf16, kind="ExternalOutput")

    with ExitStack() as ctx:
        xb = ctx.enter_context(nc.sbuf_tensor("xb", [P, 8 * sumT], bf16))
        wb = [
            ctx.enter_context(
                nc.sbuf_tensor(f"wb{b}", [P, KC * D], f8 if is8[b] else bf16)
            )
            for b in range(WBUF)
        ]
        ob = [
            ctx.enter_context(nc.sbuf_tensor(f"ob{i}", [P, 8 * max(prof)], bf16))
            for i in range(OB)
        ]
        # Warm-up operands are never initialized: the PE computes on
        # whatever SBUF holds; results land in pb[7] and are reset by the
        # first real f=7 accumulation group (start=True).
        warm = ctx.enter_context(nc.sbuf_tensor("warmt", [P, P + 512], bf16))
        pb = [
            ctx.enter_context(nc.psum_tensor(f"pb{i}", [P, 512], f32))
            for i in range(8)
        ]
        sem_x = [ctx.enter_context(nc.semaphore(f"sem_x{i}")) for i in range(S)]
        # One sem per sub-DMA: two DMAs incrementing one lane can
        # interleave their 16 per-engine increments, so a >=16 wait could
        # pass while the first one is still partially in flight.
        sem_w = [
            ctx.enter_context(nc.semaphore(f"sem_w{i}")) for i in range(wbase[S])
        ]
        sem_o = [
            ctx.enter_context(nc.semaphore(f"sem_o{i}")) for i in range(OB)
        ]
        # dedicated sem for the split second half of the final store
        sem_ol = ctx.enter_context(nc.semaphore("sem_ol"))
        sem_mm = ctx.enter_context(nc.semaphore("sem_mm"))  # f-groups done
        sem_cp = ctx.enter_context(nc.semaphore("sem_cp"))  # vector casts

        sems = sem_x + sem_w + sem_o + [sem_ol, sem_mm, sem_cp]
        nums = sorted(sm.num for sm in sems)
        # Prior programs (e.g. the XLA router executables) leave
        # semaphores dirty: clear ours on gpsimd while every engine holds
        # at an NRT-level pseudo-barrier, then start.
        nc.gpsimd.dma_reset(range(nums[0], nums[-1] + 1))
        nc._nrt_pseudo_barrier()

        block = ctx.enter_context(nc.Block())

        @block.sync
        def _(sync):
            # The whole input stream rides ONE ring in consumption order:
            # x_j then slot j's two weight half-DMAs (f-chunks 0-3 /
            # 4-7, so the PE can start a slot one half early). FIFO
            # position IS the prefetch schedule — no gating needed, and
            # slot j's weights land ~(2.4MB * j)/rate into the stream.
            for j in range(S):
                if j > 0:  # x0 rides the scalar ring (parallel head gen)
                    sync.dma_start(
                        out=xb[:, Xoff[j] : Xoff[j + 1]],
                        in_=xT[:, Xoff[j] : Xoff[j + 1]],
                    ).then_inc(sem_x[j], 16)
                src = w8[gidx[j]] if is8[j] else w[gidx[j]]
                nq = NQ[j]
                qw = KC * D // nq
                for h in range(nq):
                    sync.dma_start(
                        out=wb[j][:, h * qw : (h + 1) * qw],
                        in_=src[:, h * qw : (h + 1) * qw],
                    ).then_inc(sem_w[wbase[j] + h], 16)
            for l in range(OB):
                uses = (S - l + OB - 1) // OB
                if uses > 0:
                    sync.wait_ge(sem_o[l], 16 * uses)
            sync.wait_ge(sem_ol, 16)

        @block.scalar
        def _(scalar):
            # x0 head load (parallel with sync's W gen), then out stores
            # gated on the slot's casts (sem_cp counts 2 per slot).
            scalar.dma_start(
                out=xb[:, Xoff[0] : Xoff[1]], in_=xT[:, Xoff[0] : Xoff[1]]
            ).then_inc(sem_x[0], 16)
            for j in range(S):
                wj = 8 * prof[j]
                if j == S - 1:
                    # split the final store: first half overlaps the last
                    # matmul groups + casts, shortening the tail
                    scalar.wait_ge(sem_cp, 2 * j + 1)
                    scalar.dma_start(
                        out=out[:, Xoff[j] : Xoff[j] + wj // 2],
                        in_=ob[j % OB][:, : wj // 2],
                    ).then_inc(sem_o[j % OB], 16)
                    scalar.wait_ge(sem_cp, 2 * j + 2)
                    scalar.dma_start(
                        out=out[:, Xoff[j] + wj // 2 : Xoff[j] + wj],
                        in_=ob[j % OB][:, wj // 2 : wj],
                    ).then_inc(sem_ol, 16)
                else:
                    scalar.wait_ge(sem_cp, 2 * (j + 1))
                    scalar.dma_start(
                        out=out[:, Xoff[j] : Xoff[j] + wj], in_=ob[j % OB][:, :wj]
                    ).then_inc(sem_o[j % OB], 16)

        @block.tensor
        def _(tensor):
            # Garbage warm-up matmuls (N=512) bridge the DMA head so the
            # HAM clock gate (1.2->2.4GHz) is released when real work
            # starts, AND park the PE's inevitable stream-lag idle at the
            # head (any mid-stream idle risks a >3.4us HAM re-throttle).
            for _ in range(WARM):
                nc.tensor.matmul(
                    pb[7][:, 0:512], warm[:, :P], warm[:, P : P + 512],
                    start=True, stop=True,
                )
            # Semaphore ops are kept rare: each wait/inc instruction on an
            # engine queue costs ~115ns of serial EVENT_SEMAPHORE drain at
            # block exit, so coarse gating (per half-slot) beats per-group.
            for j in range(S):
                Tj = prof[j]
                fq = FC // NQ[j]  # f-chunks per weight sub-DMA
                for f in range(FC):
                    if f == 0:
                        tensor.wait_ge(sem_x[j], 16)
                        if j >= 1:
                            # psum banks 0-3 reused from slot j-1: wait for
                            # its first cast-half
                            tensor.wait_ge(sem_cp, 2 * (j - 1) + 1)
                    if f == 4 and j >= 1:
                        tensor.wait_ge(sem_cp, 2 * (j - 1) + 2)
                    if f % fq == 0:
                        tensor.wait_ge(sem_w[wbase[j] + f // fq], 16)
                    for kk in range(KC):
                        mm = nc.tensor.matmul(
                            pb[f][:, 0:Tj],
                            wb[j][
                                :, f * 1024 + kk * 128 : f * 1024 + (kk + 1) * 128
                            ],
                            xb[:, Xoff[j] + kk * Tj : Xoff[j] + (kk + 1) * Tj],
                            start=(kk == 0),
                            stop=(kk == KC - 1),
                        )
                    if f % 2 == 1:
                        mm.then_inc(sem_mm, 1)  # counts 4 per slot

        @block.vector
        def _(vector):
            for j in range(S):
                Tj = prof[j]
                for f in range(FC):
                    if f % 2 == 0:
                        # covers groups f and f+1 of slot j
                        vector.wait_ge(sem_mm, 4 * j + f // 2 + 1)
                    if j >= OB and f == 0:
                        vector.wait_ge(sem_o[j % OB], 16 * (j // OB))
                    cp = nc.vector.tensor_copy(
                        ob[j % OB][:, f * Tj : (f + 1) * Tj], pb[f][:, 0:Tj]
                    )
                    if f % 4 == 3:
                        cp.then_inc(sem_cp, 1)  # counts 2 per slot

    return nc


def kernel(x, Wr, br, We, be):
    global _last_results
    x = np.ascontiguousarray(np.asarray(x, dtype=np.float32))
    Wr = np.asarray(Wr, dtype=np.float32)
    br = np.asarray(br, dtype=np.float32)
    We = np.asarray(We, dtype=np.float32)
    be = np.asarray(be, dtype=np.float32)

    idx = _route(x, Wr, br)  # [N, 2] int32
    pr = np.sort(idx, axis=1)
    pid_tok = pr[:, 0] * E + pr[:, 1]  # pair id per token

    order = np.argsort(pid_tok, kind="stable")
    pids, starts = np.unique(pid_tok[order], return_index=True)
    tok_lists = np.split(order, starts[1:])
    pieces = list(zip(pids.tolist(), tok_lists))

    grid, prof = _pack(pieces)
    S = len(prof)

    # The n8 token-smallest ranks are stored fp8e4 (x64 host scale,
    # undone on host after gather): err ~ 0.0265*sqrt(token fraction) —
    # 2 ranks keep a ~30% margin under the 2e-2 gate. Compute order
    # interleaves fp8 (1MB) and bf16 (2MB) slots, smallest-first, so the
    # weight-stream pace matches the PE pace and the head gate is small.
    n8 = int(os.environ.get("KERNEL_FP8_RANKS", "3"))
    n8 = max(0, min(n8, S - 1))
    f8_ranks = list(range(S - n8, S))  # prof is sorted desc
    bf_ranks = list(range(S - n8))
    # fp8 slots (PE-heavy per byte) first, smallest first: short head
    # gate, PE backlog builds early, then the bf16 stream drains it.
    perm = f8_ranks[::-1] + bf_ranks
    grid = [[grid[c][r] for r in perm] for c in range(NCORES)]
    is8 = tuple(r in f8_ranks for r in perm)
    prof = [prof[r] for r in perm]
    nbf = S - n8
    sumT = sum(prof)
    Xoff = np.concatenate([[0], np.cumsum([8 * t for t in prof])])
    F8 = ml_dtypes.float8_e4m3
    F8_SCALE = 64.0

    x_bf = x.astype(BF16)
    wp_cache = {}

    def wmat(pid, fp8):
        """Merged pair weight in [128, f*1024 + kk*128 + c] layout."""
        if (pid, fp8) not in wp_cache:
            e1, e2 = pid // E, pid % E
            Wp = 0.5 * (We[e1] + We[e2])
            if fp8:
                Wp = (Wp * F8_SCALE).astype(F8)
            else:
                Wp = Wp.astype(BF16)
            wp_cache[(pid, fp8)] = np.ascontiguousarray(
                Wp.reshape(KC, P, FC, P).transpose(1, 2, 0, 3).reshape(P, KC * D)
            )
        return wp_cache[(pid, fp8)]

    gidx = []
    c_bf = c_f8 = 0
    for r in range(S):
        gidx.append(c_f8 if is8[r] else c_bf)
        c_f8 += is8[r]
        c_bf += not is8[r]

    xT_cores = np.zeros((NCORES, P, 8 * sumT), dtype=BF16)
    w_cores = np.zeros((NCORES, max(nbf, 1), P, KC * D), dtype=BF16)
    w8_cores = np.zeros((NCORES, max(n8, 1), P, KC * D), dtype=F8)
    for c in range(NCORES):
        for r in range(S):
            pid, toks = grid[c][r]
            if pid < 0:
                continue
            if is8[r]:
                w8_cores[c, gidx[r]] = wmat(pid, True)
            else:
                w_cores[c, gidx[r]] = wmat(pid, False)
            Tr = prof[r]
            xs = np.zeros((Tr, D), dtype=BF16)
            xs[: len(toks)] = x_bf[toks]
            # [128, kk*Tr + t] = x[tok_t, kk*128 + p]
            blk = xs.reshape(Tr, KC, P).transpose(2, 1, 0).reshape(P, 8 * Tr)
            xT_cores[c, :, Xoff[r] : Xoff[r + 1]] = blk

    key = (tuple(prof), is8)
    if key not in _prog_cache:
        _prog_cache[key] = _build_program(prof, is8)
    nc = _prog_cache[key]

    in_maps = [{"xT": xT_cores[c], "w": w_cores[c]} for c in range(NCORES)]
    if n8:
        for c in range(NCORES):
            in_maps[c]["w8"] = w8_cores[c]
    res = run_bass_kernel_spmd(nc, in_maps, core_ids=list(range(NCORES)))
    _last_results = res

    y = np.zeros((N, D), dtype=np.float32)
    covered = np.zeros(N, dtype=np.int64)
    for c in range(NCORES):
        oc = res.results[c]["out"]
        for r in range(S):
            pid, toks = grid[c][r]
            if pid < 0 or len(toks) == 0:
                continue
            Tr = prof[r]
            blk = oc[:, Xoff[r] : Xoff[r + 1]].reshape(P, FC, Tr)
            ys = blk.transpose(2, 1, 0).reshape(Tr, D)[: len(toks)]
            ys = ys.astype(np.float32)
            if is8[r]:
                ys /= F8_SCALE
            e1, e2 = pid // E, pid % E
            y[toks] = ys + 0.5 * (be[e1] + be[e2])
            covered[toks] += 1

    assert (covered == 1).all(), "dispatch did not cover every token once"
    return y
